# revision 1
# baseline (speedup 1.0000x reference)
# Trainium2 Bass kernel for nn_Decoder (dense transformer decoder).
# Data-parallel over batch across 8 NeuronCores; weights replicated.
import numpy as np
import ml_dtypes
from contextlib import ExitStack

import concourse.bass as bass
import concourse.tile as tile
from concourse import bacc, mybir
from concourse.bass_utils import run_bass_kernel_spmd

F32 = mybir.dt.float32
F32R = mybir.dt.float32r
AF = mybir.ActivationFunctionType
ALU = mybir.AluOpType

B, S, L, D, H, F, NB, MLP = 128, 128, 31, 384, 8, 24, 6, 1536
NCHAR, PAD, SOS = 62, 59, 60
EPS = 1e-6
NCORES = 8
NBC = B // NCORES          # batch elems per core (16)
DC = D // 128              # d chunks (3)
MC = MLP // 128            # mlp chunks (12)
HPAD = 256                 # heads padded to 32-stride (8*32)
NEG = -1e9

# knobs
USE_F32R = True            # float32r for the big (N>=256) matmuls
SUBB = 4                   # batch sub-round size for attention buffers
MQ = 4                     # batch elems per MLP hT group (N = MQ*128)
AT_DMA = False             # transpose A via DMA engines instead of PE
BF16 = mybir.dt.bfloat16


MMDT = F32R if USE_F32R else F32


def _r(ap):
    """matmul-input tiles already carry MMDT"""
    return ap


def build_nc(nb, trivial_ln, repeat=1):
    """Build the Bass module for one core processing nb batch elements."""
    nc = bacc.Bacc("TRN2", target_bir_lowering=False, debug=False,
                   num_devices=NCORES)

    enc_d = nc.dram_tensor("enc", [nb, 128, D], F32, kind="ExternalInput")
    oh_d = nc.dram_tensor("oh", [NCHAR, nb * 128], F32, kind="ExternalInput")
    cst_d = nc.dram_tensor("cst", [128, 128 + 128 + D + D], F32,
                           kind="ExternalInput")
    # per-block weight blobs (index 0 = pmha, 1..6 = blocks)
    qkv_d = nc.dram_tensor("qkv", [NB + 1, 128, 3 * DC * HPAD], MMDT,
                           kind="ExternalInput")
    wow1_d = nc.dram_tensor("wow1", [NB + 1, 128, 2 * D + DC * MLP], MMDT,
                            kind="ExternalInput")
    w2_d = nc.dram_tensor("w2", [NB, 128, MC * D], MMDT, kind="ExternalInput")
    lnw_d = None
    if not trivial_ln:
        lnw_d = nc.dram_tensor("lnw", [NB + 1, 128, 4 * D], F32,
                               kind="ExternalInput")
    zpad_d = nc.dram_tensor("zpad", [128, 128], MMDT, kind="ExternalInput")
    identb_d = nc.dram_tensor("identb", [128, 128], BF16,
                              kind="ExternalInput")
    out_d = nc.dram_tensor("out", [nb, L, D], F32, kind="ExternalOutput")

    W = nb * 128               # width of a per-batch-major buffer

    with tile.TileContext(nc) as tc, ExitStack() as ctx:
        cpool = ctx.enter_context(tc.tile_pool(name="cpool", bufs=1))
        xpool = ctx.enter_context(tc.tile_pool(name="xpool", bufs=1))
        ktpool = ctx.enter_context(tc.tile_pool(name="ktpool", bufs=1))
        wq_pool = ctx.enter_context(tc.tile_pool(name="wq", bufs=2))
        ww_pool = ctx.enter_context(tc.tile_pool(name="ww", bufs=1))
        w2_pool = ctx.enter_context(tc.tile_pool(name="w2", bufs=1))
        ln_pool = ctx.enter_context(tc.tile_pool(name="ln", bufs=2))
        xt_pool = ctx.enter_context(tc.tile_pool(name="xt", bufs=1))
        qk_pool = ctx.enter_context(tc.tile_pool(name="qk", bufs=1))
        sc_pool = ctx.enter_context(tc.tile_pool(name="sc", bufs=2))
        ot_pool = ctx.enter_context(tc.tile_pool(name="ot", bufs=2))
        st_pool = ctx.enter_context(tc.tile_pool(name="st", bufs=8))
        rs_pool = ctx.enter_context(tc.tile_pool(name="rs", bufs=3))
        ht_pool = ctx.enter_context(tc.tile_pool(name="ht", bufs=MC))
        eb_pool = ctx.enter_context(tc.tile_pool(name="eb", bufs=2))
        ps = ctx.enter_context(tc.tile_pool(name="ps", bufs=8, space="PSUM"))

        # ---- constants ----
        cst = cpool.tile([128, 128 + 128 + D + D], F32, tag="cst", name="cst")
        nc.gpsimd.dma_start(cst[:], cst_d[:])
        ident = cst[:, 0:128]
        maskb = cst[:, 128:256]
        pos = cst[:, 256:256 + D]
        cemb = cst[:, 256 + D:256 + 2 * D]

        eps_t = cpool.tile([128, 1], F32, tag="eps", name="eps")
        nc.gpsimd.memset(eps_t[:], EPS)
        identb = cpool.tile([128, 128], BF16, tag="identb", name="identb")
        nc.gpsimd.dma_start(identb[:], identb_d[:])

        # persistent x tiles, encT (chunk-major [128, DC*W])
        x = [xpool.tile([128, D], F32, tag=f"x{b}", name=f"x{b}")
             for b in range(nb)]
        encT = ktpool.tile([128, DC * W], MMDT, tag="encT", name="encT")

        def tpose_batch(dst, b, src):
            """transpose src [128, D] into dst [128, DC*W] at column b"""
            p = ps.tile([128, D], F32, tag="ps", name="ps")
            for c in range(DC):
                nc.tensor.transpose(p[:, 128 * c:128 * (c + 1)],
                                    src[:, 128 * c:128 * (c + 1)], ident)
            d3 = dst.rearrange("p (c s) -> p c s", c=DC)
            nc.vector.tensor_copy(
                d3[:, :, 128 * b:128 * (b + 1)],
                p.rearrange("p (c s) -> p c s", c=DC))

        for _rep in range(repeat):
            # ---- embedding: x0 = pos + char_emb[phrase] ----
            if True:
                oh = xt_pool.tile([NCHAR, W], F32, tag="xt", name="oh")
                nc.gpsimd.dma_start(oh[:], oh_d[:])
                for b in range(nb):
                    p = ps.tile([128, D], F32, tag="ps", name="ps")
                    nc.tensor.matmul(p[:],
                                     oh[0:NCHAR, 128 * b:128 * (b + 1)],
                                     cemb[0:NCHAR, :])
                    nc.vector.scalar_tensor_tensor(
                        x[b][:], p[:], 0.0, pos, op0=ALU.bypass, op1=ALU.add)

            # ---- transpose encoder outputs into encT ----
            for b in range(nb):
                et = eb_pool.tile([128, D], F32, tag="eb", name="eb")
                nc.gpsimd.dma_start(et[:], enc_d[b, :, :])
                tpose_batch(encT, b, et)

            # ---- blocks ----
            for blk in range(NB + 1):
                has_mlp = blk > 0
                qkvw = wq_pool.tile([128, 3 * DC * HPAD], MMDT, tag="qkvw",
                                    name="qkvw")
                nc.gpsimd.dma_start(qkvw[:], qkv_d[blk, :, :])
                wq = [qkvw[:, HPAD * c:HPAD * (c + 1)] for c in range(DC)]
                wk = [qkvw[:, HPAD * (DC + c):HPAD * (DC + c + 1)]
                      for c in range(DC)]
                wv = [qkvw[:, HPAD * (2 * DC + c):HPAD * (2 * DC + c + 1)]
                      for c in range(DC)]
                wow1 = ww_pool.tile([128, 2 * D + DC * MLP], MMDT, tag="wow1",
                                    name="wow1")
                nc.gpsimd.dma_start(wow1[:], wow1_d[blk, :, :])
                wo = [wow1[:, D * m:D * (m + 1)] for m in range(2)]
                w1 = [wow1[:, 2 * D + MLP * c:2 * D + MLP * (c + 1)]
                      for c in range(DC)]
                if has_mlp:
                    w2t = w2_pool.tile([128, MC * D], MMDT, tag="w2t",
                                       name="w2t")
                    nc.gpsimd.dma_start(w2t[:], w2_d[blk - 1, :, :])
                    w2 = [w2t[:, D * m:D * (m + 1)] for m in range(MC)]
                if lnw_d is not None:
                    lnw = ln_pool.tile([128, 4 * D], F32, tag="lnw",
                                       name="lnw")
                    nc.gpsimd.dma_start(lnw[:], lnw_d[blk, :, :])

                def ln_residual(src_ps, resid):
                    res = rs_pool.tile([128, D], F32, tag="rs", name="rs")
                    nc.vector.scalar_tensor_tensor(
                        res[:], src_ps[:], 0.0, resid,
                        op0=ALU.bypass, op1=ALU.add)
                    return res

                def ln_group(jobs, which):
                    """jobs: list of (dst, res). Grouped so the ACT Sqrt
                    ops are adjacent (one function-table load)."""
                    stats = []
                    for _, res in jobs:
                        st6 = st_pool.tile([128, 6], F32, tag="st6",
                                           name="st6")
                        nc.vector.bn_stats(st6[:], res[:])
                        st2 = st_pool.tile([128, 2], F32, tag="st2",
                                           name="st2")
                        nc.vector.bn_aggr(st2[:], st6[:])
                        stats.append(st2)
                    sds = []
                    for st2 in stats:
                        sd = st_pool.tile([128, 1], F32, tag="sd", name="sd")
                        nc.scalar.activation(sd[:], st2[:, 1:2], AF.Sqrt,
                                             bias=eps_t[:])
                        sds.append(sd)
                    for (dst, res), st2, sd in zip(jobs, stats, sds):
                        rstd = st_pool.tile([128, 1], F32, tag="rstd",
                                            name="rstd")
                        nc.vector.reciprocal(rstd[:], sd[:])
                        if lnw_d is None:
                            nc.gpsimd.tensor_scalar(
                                dst, res[:], st2[:, 0:1], rstd[:],
                                op0=ALU.subtract, op1=ALU.mult)
                        else:
                            g = lnw[:, (2 * which) * D:(2 * which + 1) * D]
                            bb = lnw[:, (2 * which + 1) * D:
                                     (2 * which + 2) * D]
                            tmp = rs_pool.tile([128, D], F32, tag="rs",
                                               name="rs")
                            nc.vector.tensor_scalar(
                                tmp[:], res[:], st2[:, 0:1], rstd[:],
                                op0=ALU.subtract, op1=ALU.mult)
                            tmp2 = rs_pool.tile([128, D], F32, tag="rs",
                                                name="rs")
                            nc.vector.scalar_tensor_tensor(
                                tmp2[:], tmp[:], 0.0, g,
                                op0=ALU.bypass, op1=ALU.mult)
                            nc.vector.scalar_tensor_tensor(
                                dst, tmp2[:], 0.0, bb,
                                op0=ALU.bypass, op1=ALU.add)

                # xT of current x (query input; kv input too for pmha)
                xt = xt_pool.tile([128, DC * W], MMDT, tag="xt", name="xt")
                for b in range(nb):
                    tpose_batch(xt, b, x[b])
                kvT = xt if blk == 0 else encT
                xt3 = xt.rearrange("p (c s) -> p c s", c=DC)
                kv3 = kvT.rearrange("p (c s) -> p c s", c=DC)

                # ---- attention in sub-rounds of SUBB batches ----
                for r0 in range(0, nb, SUBB):
                    nsub = min(SUBB, nb - r0)
                    w = nsub * 128
                    qt = [qk_pool.tile([128, SUBB * 128], MMDT, tag=f"qt{m}",
                                       name=f"qt{m}") for m in range(2)]
                    kt = [qk_pool.tile([128, SUBB * 128 + 128], MMDT,
                                       tag=f"kt{m}", name=f"kt{m}")
                          for m in range(2)]
                    for m in range(2):
                        nc.gpsimd.dma_start(kt[m][:, w:w + 128], zpad_d[:])
                        for n0 in range(0, w, 512):
                            nn = min(512, w - n0)
                            p = ps.tile([128, 512], F32, tag="ps", name="ps")
                            for c in range(DC):
                                nc.tensor.matmul(
                                    p[:, :nn],
                                    _r(wq[c][:, 128 * m:128 * (m + 1)]),
                                    _r(xt3[:, c, r0 * 128 + n0:
                                           r0 * 128 + n0 + nn]),
                                    start=(c == 0), stop=(c == DC - 1))
                            nc.vector.tensor_copy(qt[m][:, n0:n0 + nn],
                                                  p[:, :nn])
                            p2 = ps.tile([128, 512], F32, tag="ps", name="ps")
                            for c in range(DC):
                                nc.tensor.matmul(
                                    p2[:, :nn],
                                    _r(wk[c][:, 128 * m:128 * (m + 1)]),
                                    _r(kv3[:, c, r0 * 128 + n0:
                                           r0 * 128 + n0 + nn]),
                                    start=(c == 0), stop=(c == DC - 1))
                            nc.vector.tensor_copy(kt[m][:, n0:n0 + nn],
                                                  p2[:, :nn])
                    vt = qk_pool.tile([128, SUBB * HPAD], BF16, tag="vt",
                                      name="vt")
                    for bi in range(nsub):
                        b = r0 + bi
                        p = ps.tile([128, HPAD], F32, tag="ps", name="ps")
                        for c in range(DC):
                            nc.tensor.matmul(
                                p[:], _r(kv3[:, c, 128 * b:128 * (b + 1)]),
                                _r(wv[c]), start=(c == 0), stop=(c == DC - 1))
                        nc.vector.tensor_copy(
                            vt[:, HPAD * bi:HPAD * (bi + 1)], p[:])

                    ln_jobs = []
                    for bi in range(nsub):
                        b = r0 + bi
                        # scores for all heads -> sc_all [128, H*128]
                        sc_all = sc_pool.tile([128, H * 128], F32, tag="sc",
                                              name="sc")
                        for h in range(H):
                            m, j = h // 4, h % 4
                            sp = ps.tile([128, 256], F32, tag="ps", name="ps")
                            nc.tensor.matmul(
                                sp[:],
                                _r(qt[m][32 * j:32 * j + F,
                                         128 * bi:128 * (bi + 1)]),
                                _r(kt[m][32 * j:32 * j + F,
                                         128 * bi:128 * bi + 256]),
                                tile_position=(32 * j, 0))
                            nc.scalar.activation(
                                sc_all[:, 128 * h:128 * (h + 1)],
                                sp[:, :128], AF.Exp)
                        s8 = st_pool.tile([128, H], F32, tag="s8", name="s8")
                        for h in range(H):
                            nc.vector.scalar_tensor_tensor(
                                sc_all[:, 128 * h:128 * (h + 1)],
                                sc_all[:, 128 * h:128 * (h + 1)],
                                0.0, maskb, op0=ALU.bypass, op1=ALU.mult,
                                accum_out=s8[:, h:h + 1])
                        r8 = st_pool.tile([128, H], F32, tag="r8", name="r8")
                        nc.vector.reciprocal(r8[:], s8[:])
                        # normalize -> bf16 A
                        at_src = sc_pool.tile([128, H * 128], BF16,
                                              tag="atsrc", name="atsrc")
                        for h in range(H):
                            nc.gpsimd.tensor_scalar_mul(
                                at_src[:, 128 * h:128 * (h + 1)],
                                sc_all[:, 128 * h:128 * (h + 1)],
                                r8[:, h:h + 1])
                        # transpose A -> at_all (bf16)
                        at_all = ot_pool.tile([128, H * 128], BF16,
                                              tag="at", name="at")
                        if AT_DMA:
                            for h in range(H):
                                nc.sync.dma_start_transpose(
                                    at_all[:, 128 * h:128 * (h + 1)],
                                    at_src[:, 128 * h:128 * (h + 1)])
                        else:
                            for m in range(2):
                                p = ps.tile([128, 512], BF16, tag="ps",
                                            name="ps", padded_shape=[128, 1024])
                                for j in range(4):
                                    h = 4 * m + j
                                    nc.tensor.transpose(
                                        p[:, 128 * j:128 * (j + 1)],
                                        at_src[:, 128 * h:128 * (h + 1)],
                                        identb)
                                nc.vector.tensor_copy(
                                    at_all[:, 512 * m:512 * (m + 1)], p[:])
                        # outT: col-packed A@V (bf16) -> [hf(pad32), s]
                        ot_ps = [ps.tile([128, 128], F32, tag="ps", name="ps")
                                 for _ in range(2)]
                        for h in range(H):
                            m, j = h // 4, h % 4
                            nc.tensor.matmul(
                                ot_ps[m][32 * j:32 * j + 32, :],
                                vt[:, HPAD * bi + 32 * h:
                                   HPAD * bi + 32 * (h + 1)],
                                at_all[:, 128 * h:128 * (h + 1)],
                                tile_position=(0, 32 * j))
                        otsb = [ot_pool.tile([128, 128], MMDT, tag=f"ot{m}",
                                             name=f"ot{m}") for m in range(2)]
                        for m in range(2):
                            nc.vector.tensor_copy(otsb[m][:], ot_ps[m][:])
                        # wo projection + residual; LN deferred to group
                        wp = ps.tile([128, D], F32, tag="ps", name="ps")
                        for m in range(2):
                            nc.tensor.matmul(wp[:], _r(otsb[m][:]), _r(wo[m]),
                                             start=(m == 0), stop=(m == 1))
                        ln_jobs.append((x[b][:], ln_residual(wp, x[b][:])))
                    ln_group(ln_jobs, 0)

                # ---- MLP ----
                if has_mlp:
                    xt2 = xt_pool.tile([128, DC * W], MMDT, tag="xt",
                                       name="xt2")
                    for b in range(nb):
                        tpose_batch(xt2, b, x[b])
                    xt23 = xt2.rearrange("p (c s) -> p c s", c=DC)
                    for g0 in range(0, nb, MQ):
                        ng = min(MQ, nb - g0)
                        wg = ng * 128
                        hts = []
                        for m in range(MC):
                            p = ps.tile([128, MQ * 128], F32, tag="ps",
                                        name="ps")
                            for c in range(DC):
                                nc.tensor.matmul(
                                    p[:, :wg],
                                    _r(w1[c][:, 128 * m:128 * (m + 1)]),
                                    _r(xt23[:, c, 128 * g0:128 * g0 + wg]),
                                    start=(c == 0), stop=(c == DC - 1))
                            ht = ht_pool.tile([128, MQ * 128], MMDT, tag="ht",
                                              name="ht")
                            nc.scalar.activation(ht[:, :wg], p[:, :wg],
                                                 AF.Gelu)
                            hts.append(ht)
                        ln_jobs = []
                        for bi in range(ng):
                            b = g0 + bi
                            p = ps.tile([128, D], F32, tag="ps", name="ps")
                            for m in range(MC):
                                nc.tensor.matmul(
                                    p[:],
                                    _r(hts[m][:, 128 * bi:128 * (bi + 1)]),
                                    _r(w2[m]),
                                    start=(m == 0), stop=(m == MC - 1))
                            ln_jobs.append(
                                (x[b][:], ln_residual(p, x[b][:])))
                        ln_group(ln_jobs, 1)

        # ---- output ----
        for b in range(nb):
            nc.gpsimd.dma_start(out_d[b, :, :], x[b][0:L, :])

    nc.compile()
    return nc


def prep_host(inputs, nb_per_core=NBC, ncores=NCORES):
    """Host-side layout prep. Returns (in_maps, trivial_ln)."""
    f32 = np.float32
    enc = np.asarray(inputs["encoder_outputs"], f32)
    phrase = np.asarray(inputs["phrase"]).astype(np.int64)

    # padded phrase -> one-hot (transposed) per batch elem
    ph = np.full((B, S), PAD, np.int64)
    ph[:, 0] = SOS
    ph[:, 1:1 + L] = phrase
    ohT = np.zeros((B, NCHAR, S), f32)
    bidx = np.arange(B)[:, None]
    sidx = np.arange(S)[None, :]
    ohT[bidx, ph, sidx] = 1.0

    scale = 1.0 / np.sqrt(np.float32(F))

    def pad32(w):  # [H, D, F] -> [D, HPAD] with heads at 32-stride
        out = np.zeros((D, HPAD), f32)
        for h in range(H):
            out[:, 32 * h:32 * h + F] = w[h]
        return out

    def wo_pad(w):  # [H*F, D] -> [HPAD, D] rows at 32-stride
        out = np.zeros((HPAD, D), f32)
        for h in range(H):
            out[32 * h:32 * h + F] = w[F * h:F * (h + 1)]
        return out

    def chunks(w, p=128):  # [K, N] -> [128, (K//128)*N] side by side
        k = w.shape[0]
        return np.concatenate([w[i:i + p] for i in range(0, k, p)], axis=1)

    qkv_blob = np.zeros((NB + 1, 128, 3 * DC * HPAD), f32)
    wow1_blob = np.zeros((NB + 1, 128, 2 * D + DC * MLP), f32)
    w2_blob = np.zeros((NB, 128, MC * D), f32)
    lnw_blob = np.zeros((NB + 1, 128, 4 * D), f32)

    def fill_blk(i, wq, wk, wv, wo, w1=None, w2=None):
        qkv_blob[i, :, 0:DC * HPAD] = chunks(pad32(np.asarray(wq, f32)) * scale)
        qkv_blob[i, :, DC * HPAD:2 * DC * HPAD] = chunks(
            pad32(np.asarray(wk, f32)))
        qkv_blob[i, :, 2 * DC * HPAD:] = chunks(pad32(np.asarray(wv, f32)))
        wow1_blob[i, :, 0:2 * D] = chunks(wo_pad(np.asarray(wo, f32)))
        if w1 is not None:
            wow1_blob[i, :, 2 * D:] = chunks(np.asarray(w1, f32))
            w2_blob[i - 1] = chunks(np.asarray(w2, f32))

    fill_blk(0, inputs["pmha_wq"], inputs["pmha_wk"], inputs["pmha_wv"],
             inputs["pmha_wo"])
    for i in range(NB):
        fill_blk(i + 1, inputs["blk_wq"][i], inputs["blk_wk"][i],
                 inputs["blk_wv"][i], inputs["blk_wo"][i],
                 inputs["mlp_w1"][i], inputs["mlp_w2"][i])

    lns = [(inputs["pln_g"], inputs["pln_b"], None, None)] + [
        (inputs["ln1_g"][i], inputs["ln1_b"][i],
         inputs["ln2_g"][i], inputs["ln2_b"][i]) for i in range(NB)]
    trivial_ln = True
    for i, (g1, b1, g2, b2) in enumerate(lns):
        for which, (g, bb) in enumerate([(g1, b1), (g2, b2)]):
            if g is None:
                continue
            g = np.asarray(g, f32)
            bb = np.asarray(bb, f32)
            if not (np.all(g == 1.0) and np.all(bb == 0.0)):
                trivial_ln = False
            lnw_blob[i, :, (2 * which) * D:(2 * which + 1) * D] = g[None, :]
            lnw_blob[i, :, (2 * which + 1) * D:(2 * which + 2) * D] = bb[None, :]

    cst = np.zeros((128, 128 + 128 + D + D), f32)
    cst[:, 0:128] = np.eye(128, dtype=f32)
    mask = np.ones((S, S), f32)
    mask[np.triu_indices(S, 1)] = 0.0          # [q, k]: k > q masked
    cst[:, 128:256] = mask
    cst[:, 256:256 + D] = np.asarray(inputs["pos_emb"], f32)
    cst[0:NCHAR, 256 + D:256 + 2 * D] = np.asarray(inputs["char_emb"], f32)

    in_maps = []
    for c in range(ncores):
        b0 = c * nb_per_core
        m = {
            "enc": np.ascontiguousarray(enc[b0:b0 + nb_per_core]),
            "oh": np.ascontiguousarray(
                ohT[b0:b0 + nb_per_core].transpose(1, 0, 2).reshape(
                    NCHAR, nb_per_core * S)),
            "cst": cst,
            "qkv": qkv_blob,
            "wow1": wow1_blob,
            "w2": w2_blob,
            "zpad": np.zeros((128, 128), f32),
            "identb": np.eye(128).astype(ml_dtypes.bfloat16),
        }
        if not trivial_ln:
            m["lnw"] = lnw_blob
        in_maps.append(m)
    return in_maps, trivial_ln


def kernel(**inputs) -> np.ndarray:
    in_maps, trivial_ln = prep_host(inputs)
    nc = build_nc(NBC, trivial_ln)
    res = run_bass_kernel_spmd(nc, in_maps, list(range(NCORES)))
    out = np.concatenate([res.results[c]["out"] for c in range(NCORES)],
                         axis=0)
    return out.astype(np.float32)



# revision 9
# speedup vs baseline: 32.3828x; 32.3828x over previous
# Trainium2 Bass kernel for nn_Decoder (dense transformer decoder).
# Data-parallel over batch across 8 NeuronCores; weights replicated.
import numpy as np
import ml_dtypes
from contextlib import ExitStack

import concourse.bass as bass
import concourse.tile as tile
from concourse import bacc, mybir
from concourse.bass_utils import run_bass_kernel_spmd

F32 = mybir.dt.float32
F32R = mybir.dt.float32r
AF = mybir.ActivationFunctionType
ALU = mybir.AluOpType

B, S, L, D, H, F, NB, MLP = 128, 128, 31, 384, 8, 24, 6, 1536
NCHAR, PAD, SOS = 62, 59, 60
EPS = 1e-6
NCORES = 8
NBC = B // NCORES          # batch elems per core (16)
DC = D // 128              # d chunks (3)
MC = MLP // 128            # mlp chunks (12)
HPAD = 256                 # heads padded to 32-stride (8*32)
NEG = -1e9

# knobs
SUBB = 4                   # batch sub-round size for attention buffers
MQ = 4                     # batch elems per MLP hT group (N = MQ*128)
AT_DMA = False             # transpose A via DMA engines instead of PE
BF16 = mybir.dt.bfloat16


MMDT = BF16                # bf16 matmuls: 1 cyc/row at any width, half DMA


def _r(ap):
    """matmul-input tiles already carry MMDT"""
    return ap


def build_nc(nb, trivial_ln, repeat=1):
    """Build the Bass module for one core processing nb batch elements."""
    nc = bacc.Bacc("TRN2", target_bir_lowering=False, debug=False,
                   num_devices=NCORES)

    enc_d = nc.dram_tensor("enc", [nb, 128, D], F32, kind="ExternalInput")
    oh_d = nc.dram_tensor("oh", [NCHAR, nb * 128], F32, kind="ExternalInput")
    cst_d = nc.dram_tensor("cst", [128, 128 + 128 + D + D], F32,
                           kind="ExternalInput")
    # per-block weight blobs (index 0 = pmha, 1..6 = blocks)
    qkv_d = nc.dram_tensor("qkv", [NB + 1, 128, 3 * DC * HPAD], MMDT,
                           kind="ExternalInput")
    wow1_d = nc.dram_tensor("wow1", [NB + 1, 128, 2 * D + DC * MLP], MMDT,
                            kind="ExternalInput")
    w2_d = nc.dram_tensor("w2", [NB, 128, MC * D], MMDT, kind="ExternalInput")
    lnw_d = None
    if not trivial_ln:
        lnw_d = nc.dram_tensor("lnw", [NB + 1, 128, 4 * D], F32,
                               kind="ExternalInput")
    identb_d = nc.dram_tensor("identb", [128, 128], BF16,
                              kind="ExternalInput")
    out_d = nc.dram_tensor("out", [nb, L, D], F32, kind="ExternalOutput")

    W = nb * 128               # width of a per-batch-major buffer

    with tile.TileContext(nc) as tc, ExitStack() as ctx:
        cpool = ctx.enter_context(tc.tile_pool(name="cpool", bufs=1))
        xpool = ctx.enter_context(tc.tile_pool(name="xpool", bufs=1))
        ktpool = ctx.enter_context(tc.tile_pool(name="ktpool", bufs=1))
        wq_pool = ctx.enter_context(tc.tile_pool(name="wq", bufs=2))
        ww_pool = ctx.enter_context(tc.tile_pool(name="ww", bufs=2))
        w2_pool = ctx.enter_context(tc.tile_pool(name="w2", bufs=2))
        ln_pool = ctx.enter_context(tc.tile_pool(name="ln", bufs=2))
        xt_pool = ctx.enter_context(tc.tile_pool(name="xt", bufs=1))
        qk_pool = ctx.enter_context(tc.tile_pool(name="qk", bufs=1))
        sc_pool = ctx.enter_context(tc.tile_pool(name="sc", bufs=2))
        ot_pool = ctx.enter_context(tc.tile_pool(name="ot", bufs=2))
        st_pool = ctx.enter_context(tc.tile_pool(name="st", bufs=8))
        rs_pool = ctx.enter_context(tc.tile_pool(name="rs", bufs=3))
        ht_pool = ctx.enter_context(tc.tile_pool(name="ht", bufs=MC))
        eb_pool = ctx.enter_context(tc.tile_pool(name="eb", bufs=2))
        ps = ctx.enter_context(tc.tile_pool(name="ps", bufs=8, space="PSUM"))

        # ---- constants ----
        cst = cpool.tile([128, 128 + 128 + D + D], F32, tag="cst", name="cst")
        nc.gpsimd.dma_start(cst[:], cst_d[:])
        ident = cst[:, 0:128]
        maskb = cst[:, 128:256]
        pos = cst[:, 256:256 + D]
        cemb = cst[:, 256 + D:256 + 2 * D]

        eps_t = cpool.tile([128, 1], F32, tag="eps", name="eps")
        nc.gpsimd.memset(eps_t[:], EPS)
        identb = cpool.tile([128, 128], BF16, tag="identb", name="identb")
        nc.gpsimd.dma_start(identb[:], identb_d[:])

        # persistent x tiles, encT (chunk-major [128, DC*W])
        x = [xpool.tile([128, D], F32, tag=f"x{b}", name=f"x{b}")
             for b in range(nb)]
        encT = ktpool.tile([128, DC * W], MMDT, tag="encT", name="encT")

        def tpose_batch(dst, b, src):
            """transpose src [128, D] into dst [128, DC*W] at column b"""
            p = ps.tile([128, D], F32, tag="ps", name="ps")
            for c in range(DC):
                nc.tensor.transpose(p[:, 128 * c:128 * (c + 1)],
                                    src[:, 128 * c:128 * (c + 1)], ident)
            d3 = dst.rearrange("p (c s) -> p c s", c=DC)
            nc.vector.tensor_copy(
                d3[:, :, 128 * b:128 * (b + 1)],
                p.rearrange("p (c s) -> p c s", c=DC))

        for _rep in range(repeat):
            # ---- embedding: x0 = pos + char_emb[phrase] ----
            if True:
                oh = xt_pool.tile([NCHAR, W], F32, tag="xt", name="oh")
                nc.gpsimd.dma_start(oh[:], oh_d[:])
                for b in range(nb):
                    p = ps.tile([128, D], F32, tag="ps", name="ps")
                    nc.tensor.matmul(p[:],
                                     oh[0:NCHAR, 128 * b:128 * (b + 1)],
                                     cemb[0:NCHAR, :])
                    nc.vector.scalar_tensor_tensor(
                        x[b][:], p[:], 0.0, pos, op0=ALU.bypass, op1=ALU.add)

            # ---- transpose encoder outputs into encT ----
            for b in range(nb):
                et = eb_pool.tile([128, D], F32, tag="eb", name="eb")
                nc.gpsimd.dma_start(et[:], enc_d[b, :, :])
                tpose_batch(encT, b, et)

            # ---- blocks ----
            for blk in range(NB + 1):
                has_mlp = blk > 0
                qkvw = wq_pool.tile([128, 3 * DC * HPAD], MMDT, tag="qkvw",
                                    name="qkvw")
                nc.gpsimd.dma_start(qkvw[:], qkv_d[blk, :, :])
                wq = [qkvw[:, HPAD * c:HPAD * (c + 1)] for c in range(DC)]
                wk = [qkvw[:, HPAD * (DC + c):HPAD * (DC + c + 1)]
                      for c in range(DC)]
                wv = [qkvw[:, HPAD * (2 * DC + c):HPAD * (2 * DC + c + 1)]
                      for c in range(DC)]
                wow1 = ww_pool.tile([128, 2 * D + DC * MLP], MMDT, tag="wow1",
                                    name="wow1")
                nc.gpsimd.dma_start(wow1[:], wow1_d[blk, :, :])
                wo = [wow1[:, D * m:D * (m + 1)] for m in range(2)]
                w1 = [wow1[:, 2 * D + MLP * c:2 * D + MLP * (c + 1)]
                      for c in range(DC)]
                if has_mlp:
                    w2t = w2_pool.tile([128, MC * D], MMDT, tag="w2t",
                                       name="w2t")
                    nc.gpsimd.dma_start(w2t[:], w2_d[blk - 1, :, :])
                    w2 = [w2t[:, D * m:D * (m + 1)] for m in range(MC)]
                if lnw_d is not None:
                    lnw = ln_pool.tile([128, 4 * D], F32, tag="lnw",
                                       name="lnw")
                    nc.gpsimd.dma_start(lnw[:], lnw_d[blk, :, :])

                def ln_residual(src_ps, resid):
                    res = rs_pool.tile([128, D], F32, tag="rs", name="rs")
                    nc.vector.scalar_tensor_tensor(
                        res[:], src_ps[:], 0.0, resid,
                        op0=ALU.bypass, op1=ALU.add)
                    return res

                def ln_group(jobs, which):
                    """jobs: list of (dst, res). Grouped so the ACT Sqrt
                    ops are adjacent (one function-table load)."""
                    stats = []
                    for _, res in jobs:
                        st6 = st_pool.tile([128, 6], F32, tag="st6",
                                           name="st6")
                        nc.vector.bn_stats(st6[:], res[:])
                        st2 = st_pool.tile([128, 2], F32, tag="st2",
                                           name="st2")
                        nc.vector.bn_aggr(st2[:], st6[:])
                        stats.append(st2)
                    sds = []
                    for st2 in stats:
                        sd = st_pool.tile([128, 1], F32, tag="sd", name="sd")
                        nc.scalar.activation(sd[:], st2[:, 1:2], AF.Sqrt,
                                             bias=eps_t[:])
                        sds.append(sd)
                    for (dst, res), st2, sd in zip(jobs, stats, sds):
                        rstd = st_pool.tile([128, 1], F32, tag="rstd",
                                            name="rstd")
                        nc.vector.reciprocal(rstd[:], sd[:])
                        if lnw_d is None:
                            nc.vector.tensor_scalar(
                                dst, res[:], st2[:, 0:1], rstd[:],
                                op0=ALU.subtract, op1=ALU.mult)
                        else:
                            g = lnw[:, (2 * which) * D:(2 * which + 1) * D]
                            bb = lnw[:, (2 * which + 1) * D:
                                     (2 * which + 2) * D]
                            tmp = rs_pool.tile([128, D], F32, tag="rs",
                                               name="rs")
                            nc.vector.tensor_scalar(
                                tmp[:], res[:], st2[:, 0:1], rstd[:],
                                op0=ALU.subtract, op1=ALU.mult)
                            tmp2 = rs_pool.tile([128, D], F32, tag="rs",
                                                name="rs")
                            nc.vector.scalar_tensor_tensor(
                                tmp2[:], tmp[:], 0.0, g,
                                op0=ALU.bypass, op1=ALU.mult)
                            nc.vector.scalar_tensor_tensor(
                                dst, tmp2[:], 0.0, bb,
                                op0=ALU.bypass, op1=ALU.add)

                # xT of current x (query input; kv input too for pmha)
                xt = xt_pool.tile([128, DC * W], MMDT, tag="xt", name="xt")
                for b in range(nb):
                    tpose_batch(xt, b, x[b])
                kvT = xt if blk == 0 else encT
                xt3 = xt.rearrange("p (c s) -> p c s", c=DC)
                kv3 = kvT.rearrange("p (c s) -> p c s", c=DC)

                # ---- attention in sub-rounds of SUBB batches ----
                for r0 in range(0, nb, SUBB):
                    nsub = min(SUBB, nb - r0)
                    w = nsub * 128
                    qt = [qk_pool.tile([128, SUBB * 128], MMDT, tag=f"qt{m}",
                                       name=f"qt{m}") for m in range(2)]
                    kt = [qk_pool.tile([128, SUBB * 128], MMDT,
                                       tag=f"kt{m}", name=f"kt{m}")
                          for m in range(2)]
                    for m in range(2):
                        for n0 in range(0, w, 512):
                            nn = min(512, w - n0)
                            p = ps.tile([128, 512], F32, tag="ps", name="ps")
                            for c in range(DC):
                                nc.tensor.matmul(
                                    p[:, :nn],
                                    _r(wq[c][:, 128 * m:128 * (m + 1)]),
                                    _r(xt3[:, c, r0 * 128 + n0:
                                           r0 * 128 + n0 + nn]),
                                    start=(c == 0), stop=(c == DC - 1))
                            nc.vector.tensor_copy(qt[m][:, n0:n0 + nn],
                                                  p[:, :nn])
                            p2 = ps.tile([128, 512], F32, tag="ps", name="ps")
                            for c in range(DC):
                                nc.tensor.matmul(
                                    p2[:, :nn],
                                    _r(wk[c][:, 128 * m:128 * (m + 1)]),
                                    _r(kv3[:, c, r0 * 128 + n0:
                                           r0 * 128 + n0 + nn]),
                                    start=(c == 0), stop=(c == DC - 1))
                            nc.vector.tensor_copy(kt[m][:, n0:n0 + nn],
                                                  p2[:, :nn])
                    vt = qk_pool.tile([128, SUBB * HPAD], BF16, tag="vt",
                                      name="vt")
                    for bi in range(nsub):
                        b = r0 + bi
                        p = ps.tile([128, HPAD], F32, tag="ps", name="ps")
                        for c in range(DC):
                            nc.tensor.matmul(
                                p[:], _r(kv3[:, c, 128 * b:128 * (b + 1)]),
                                _r(wv[c]), start=(c == 0), stop=(c == DC - 1))
                        nc.vector.tensor_copy(
                            vt[:, HPAD * bi:HPAD * (bi + 1)], p[:])

                    ln_jobs = []
                    for bi in range(nsub):
                        b = r0 + bi
                        # scores for all heads -> sc_all [128, H*128]
                        sc_all = sc_pool.tile([128, H * 128], F32, tag="sc",
                                              name="sc")
                        for h in range(H):
                            m, j = h // 4, h % 4
                            sp = ps.tile([128, 128], F32, tag="ps", name="ps")
                            nc.tensor.matmul(
                                sp[:],
                                _r(qt[m][32 * j:32 * j + F,
                                         128 * bi:128 * (bi + 1)]),
                                _r(kt[m][32 * j:32 * j + F,
                                         128 * bi:128 * (bi + 1)]),
                                tile_position=(32 * j, 0))
                            nc.scalar.activation(
                                sc_all[:, 128 * h:128 * (h + 1)],
                                sp[:, :128], AF.Exp)
                        s8 = st_pool.tile([128, H], F32, tag="s8", name="s8")
                        for h in range(H):
                            nc.vector.scalar_tensor_tensor(
                                sc_all[:, 128 * h:128 * (h + 1)],
                                sc_all[:, 128 * h:128 * (h + 1)],
                                0.0, maskb, op0=ALU.bypass, op1=ALU.mult,
                                accum_out=s8[:, h:h + 1])
                        r8 = st_pool.tile([128, H], F32, tag="r8", name="r8")
                        nc.vector.reciprocal(r8[:], s8[:])
                        # normalize -> bf16 A
                        at_src = sc_pool.tile([128, H * 128], BF16,
                                              tag="atsrc", name="atsrc")
                        for h in range(H):
                            nc.vector.tensor_scalar_mul(
                                at_src[:, 128 * h:128 * (h + 1)],
                                sc_all[:, 128 * h:128 * (h + 1)],
                                r8[:, h:h + 1])
                        # transpose A -> at_all (bf16)
                        at_all = ot_pool.tile([128, H * 128], BF16,
                                              tag="at", name="at")
                        if AT_DMA:
                            for h in range(H):
                                nc.sync.dma_start_transpose(
                                    at_all[:, 128 * h:128 * (h + 1)],
                                    at_src[:, 128 * h:128 * (h + 1)])
                        else:
                            for m in range(2):
                                p = ps.tile([128, 512], BF16, tag="ps",
                                            name="ps", padded_shape=[128, 1024])
                                for j in range(4):
                                    h = 4 * m + j
                                    nc.tensor.transpose(
                                        p[:, 128 * j:128 * (j + 1)],
                                        at_src[:, 128 * h:128 * (h + 1)],
                                        identb)
                                nc.vector.tensor_copy(
                                    at_all[:, 512 * m:512 * (m + 1)], p[:])
                        # outT: col-packed A@V (bf16) -> [hf(pad32), s]
                        ot_ps = [ps.tile([128, 128], F32, tag="ps", name="ps")
                                 for _ in range(2)]
                        for h in range(H):
                            m, j = h // 4, h % 4
                            nc.tensor.matmul(
                                ot_ps[m][32 * j:32 * j + 32, :],
                                vt[:, HPAD * bi + 32 * h:
                                   HPAD * bi + 32 * (h + 1)],
                                at_all[:, 128 * h:128 * (h + 1)],
                                tile_position=(0, 32 * j))
                        otsb = [ot_pool.tile([128, 128], MMDT, tag=f"ot{m}",
                                             name=f"ot{m}") for m in range(2)]
                        for m in range(2):
                            nc.vector.tensor_copy(otsb[m][:], ot_ps[m][:])
                        # wo projection + residual; LN deferred to group
                        wp = ps.tile([128, D], F32, tag="ps", name="ps")
                        for m in range(2):
                            nc.tensor.matmul(wp[:], _r(otsb[m][:]), _r(wo[m]),
                                             start=(m == 0), stop=(m == 1))
                        ln_jobs.append((x[b][:], ln_residual(wp, x[b][:])))
                    ln_group(ln_jobs, 0)

                # ---- MLP ----
                if has_mlp:
                    xt2 = xt_pool.tile([128, DC * W], MMDT, tag="xt",
                                       name="xt2")
                    for b in range(nb):
                        tpose_batch(xt2, b, x[b])
                    xt23 = xt2.rearrange("p (c s) -> p c s", c=DC)
                    for g0 in range(0, nb, MQ):
                        ng = min(MQ, nb - g0)
                        wg = ng * 128
                        hts = []
                        for m in range(MC):
                            p = ps.tile([128, MQ * 128], F32, tag="ps",
                                        name="ps")
                            for c in range(DC):
                                nc.tensor.matmul(
                                    p[:, :wg],
                                    _r(w1[c][:, 128 * m:128 * (m + 1)]),
                                    _r(xt23[:, c, 128 * g0:128 * g0 + wg]),
                                    start=(c == 0), stop=(c == DC - 1))
                            ht = ht_pool.tile([128, MQ * 128], MMDT, tag="ht",
                                              name="ht")
                            nc.scalar.activation(ht[:, :wg], p[:, :wg],
                                                 AF.Gelu)
                            hts.append(ht)
                        ln_jobs = []
                        for bi in range(ng):
                            b = g0 + bi
                            p = ps.tile([128, D], F32, tag="ps", name="ps")
                            for m in range(MC):
                                nc.tensor.matmul(
                                    p[:],
                                    _r(hts[m][:, 128 * bi:128 * (bi + 1)]),
                                    _r(w2[m]),
                                    start=(m == 0), stop=(m == MC - 1))
                            ln_jobs.append(
                                (x[b][:], ln_residual(p, x[b][:])))
                        ln_group(ln_jobs, 1)

        # ---- output ----
        for b in range(nb):
            nc.gpsimd.dma_start(out_d[b, :, :], x[b][0:L, :])

    nc.compile()
    return nc


def prep_host(inputs, nb_per_core=NBC, ncores=NCORES):
    """Host-side layout prep. Returns (in_maps, trivial_ln)."""
    f32 = np.float32
    enc = np.asarray(inputs["encoder_outputs"], f32)
    phrase = np.asarray(inputs["phrase"]).astype(np.int64)

    # padded phrase -> one-hot (transposed) per batch elem
    ph = np.full((B, S), PAD, np.int64)
    ph[:, 0] = SOS
    ph[:, 1:1 + L] = phrase
    ohT = np.zeros((B, NCHAR, S), f32)
    bidx = np.arange(B)[:, None]
    sidx = np.arange(S)[None, :]
    ohT[bidx, ph, sidx] = 1.0

    scale = 1.0 / np.sqrt(np.float32(F))

    def pad32(w):  # [H, D, F] -> [D, HPAD] with heads at 32-stride
        out = np.zeros((D, HPAD), f32)
        for h in range(H):
            out[:, 32 * h:32 * h + F] = w[h]
        return out

    def wo_pad(w):  # [H*F, D] -> [HPAD, D] rows at 32-stride
        out = np.zeros((HPAD, D), f32)
        for h in range(H):
            out[32 * h:32 * h + F] = w[F * h:F * (h + 1)]
        return out

    def chunks(w, p=128):  # [K, N] -> [128, (K//128)*N] side by side
        k = w.shape[0]
        return np.concatenate([w[i:i + p] for i in range(0, k, p)], axis=1)

    qkv_blob = np.zeros((NB + 1, 128, 3 * DC * HPAD), f32)
    wow1_blob = np.zeros((NB + 1, 128, 2 * D + DC * MLP), f32)
    w2_blob = np.zeros((NB, 128, MC * D), f32)
    lnw_blob = np.zeros((NB + 1, 128, 4 * D), f32)

    def fill_blk(i, wq, wk, wv, wo, w1=None, w2=None):
        qkv_blob[i, :, 0:DC * HPAD] = chunks(pad32(np.asarray(wq, f32)) * scale)
        qkv_blob[i, :, DC * HPAD:2 * DC * HPAD] = chunks(
            pad32(np.asarray(wk, f32)))
        qkv_blob[i, :, 2 * DC * HPAD:] = chunks(pad32(np.asarray(wv, f32)))
        wow1_blob[i, :, 0:2 * D] = chunks(wo_pad(np.asarray(wo, f32)))
        if w1 is not None:
            wow1_blob[i, :, 2 * D:] = chunks(np.asarray(w1, f32))
            w2_blob[i - 1] = chunks(np.asarray(w2, f32))

    fill_blk(0, inputs["pmha_wq"], inputs["pmha_wk"], inputs["pmha_wv"],
             inputs["pmha_wo"])
    for i in range(NB):
        fill_blk(i + 1, inputs["blk_wq"][i], inputs["blk_wk"][i],
                 inputs["blk_wv"][i], inputs["blk_wo"][i],
                 inputs["mlp_w1"][i], inputs["mlp_w2"][i])

    lns = [(inputs["pln_g"], inputs["pln_b"], None, None)] + [
        (inputs["ln1_g"][i], inputs["ln1_b"][i],
         inputs["ln2_g"][i], inputs["ln2_b"][i]) for i in range(NB)]
    trivial_ln = True
    for i, (g1, b1, g2, b2) in enumerate(lns):
        for which, (g, bb) in enumerate([(g1, b1), (g2, b2)]):
            if g is None:
                continue
            g = np.asarray(g, f32)
            bb = np.asarray(bb, f32)
            if not (np.all(g == 1.0) and np.all(bb == 0.0)):
                trivial_ln = False
            lnw_blob[i, :, (2 * which) * D:(2 * which + 1) * D] = g[None, :]
            lnw_blob[i, :, (2 * which + 1) * D:(2 * which + 2) * D] = bb[None, :]

    cst = np.zeros((128, 128 + 128 + D + D), f32)
    cst[:, 0:128] = np.eye(128, dtype=f32)
    mask = np.ones((S, S), f32)
    mask[np.triu_indices(S, 1)] = 0.0          # [q, k]: k > q masked
    cst[:, 128:256] = mask
    cst[:, 256:256 + D] = np.asarray(inputs["pos_emb"], f32)
    cst[0:NCHAR, 256 + D:256 + 2 * D] = np.asarray(inputs["char_emb"], f32)

    bf = ml_dtypes.bfloat16
    qkv_bf = qkv_blob.astype(bf)
    wow1_bf = wow1_blob.astype(bf)
    w2_bf = w2_blob.astype(bf)
    in_maps = []
    for c in range(ncores):
        b0 = c * nb_per_core
        m = {
            "enc": np.ascontiguousarray(enc[b0:b0 + nb_per_core]),
            "oh": np.ascontiguousarray(
                ohT[b0:b0 + nb_per_core].transpose(1, 0, 2).reshape(
                    NCHAR, nb_per_core * S)),
            "cst": cst,
            "qkv": qkv_bf,
            "wow1": wow1_bf,
            "w2": w2_bf,
            "identb": np.eye(128).astype(bf),
        }
        if not trivial_ln:
            m["lnw"] = lnw_blob
        in_maps.append(m)
    return in_maps, trivial_ln


def kernel(**inputs) -> np.ndarray:
    in_maps, trivial_ln = prep_host(inputs)
    nc = build_nc(NBC, trivial_ln)
    res = run_bass_kernel_spmd(nc, in_maps, list(range(NCORES)))
    out = np.concatenate([res.results[c]["out"] for c in range(NCORES)],
                         axis=0)
    return out.astype(np.float32)

